# revision 13
# baseline (speedup 1.0000x reference)
"""Causal multi-head attention (CoreAttention) for Trainium2, 8 NeuronCores.

Strategy
--------
The problem is 64 independent (batch, head) attention instances of
[sq=2048, hn=64].  We shard them 8-per-core (tensor-parallel over heads x
data-parallel over batch) -- fully data parallel, no collectives.

Host-side (shard prep): Q and K are pre-transposed to [pair, hn, sq] and V
gets a ones-column appended ([pair, sq, 65]), all cast to fp16 (hw-measured
rel err 3.9e-4 on the max-err/scale metric), so that on-chip:

  S^T[sk_blk, q]   = matmul(lhsT=K^T[:, blk], rhs=Q^T[:, q_chunk])    (K=hn=64)
  E = exp(S^T / 8) via ScalarE straight out of PSUM, fp16 out
  causal triangle of diagonal blocks zeroed with one DVE multiply
  ctx^T[65, q]    += matmul(lhsT=[V|1][blk], rhs=E[blk])              (K=sk=128)

ctx^T row 64 is the softmax denominator; the final division and the
transpose back to [sq, b, np*hn] happen on the host.  Skipping the max
subtraction is safe: scores/8 ~ N(0,1), |s|<~7, exp is far from overflow
even in fp16, and softmax is shift invariant so the result matches.

Causality: sk blocks strictly above the diagonal are never computed;
diagonal-band matmuls restrict their q columns to the valid range, and
their score spans are COMPACTED side by side in the PSUM staging tile
(ordered so no matmul output crosses a 512-elem PSUM bank) so each group
of blocks needs exactly one ScalarE exp instruction.

Schedule: a flat software pipeline over all (pair, chunk, group) tasks.
PV of group g is emitted after QK of group g+2 so the in-order PE never
head-of-line blocks on an exp; pair inputs are DMA-prefetched one pair
ahead; each chunk's ctx leaves PSUM via DVE copy + its own store DMA.

Per core both engine floors bind at ~116 us (ACT: 8*17408 exp columns
@1.2GHz; PE: 2*8*17408 stream cycles @2.4GHz); measured 117.2 us/iter.
"""

import os
import sys

import numpy as np

if "/opt/trn_rl_repo" not in sys.path:
    sys.path.insert(0, "/opt/trn_rl_repo")

import concourse.bass as bass
import concourse.mybir as mybir
import concourse.tile as tile
from concourse import bacc

SQ, B, NP, HN = 2048, 4, 16, 64
N_CORES = 8
PAIRS_TOTAL = B * NP            # 64 (b, h) instances
PAIRS = PAIRS_TOTAL // N_CORES  # 8 per core
CH = 512                        # q chunk (one PSUM bank of fp32)
NBLK = SQ // 128                # 16 sk blocks
GROUP = 3                       # sk blocks per PSUM score-staging tile
F32 = mybir.dt.float32
MM_DTYPE = mybir.dt.float16     # matmul operand dtype (weights + streams)


def build_attention_module(
    pairs: int = PAIRS,
    nchunks: int = SQ // CH,
    mask: bool = True,
    repeat: int = 1,
    mm_dtype=None,
    loop_n: int | None = None,
) -> bass.Bass:
    MMDT = MM_DTYPE if mm_dtype is None else mm_dtype
    nc = bacc.Bacc(trn_type="TRN2")
    qt = nc.dram_tensor("qt", [pairs, HN, SQ], MMDT, kind="ExternalInput")
    kt = nc.dram_tensor("kt", [pairs, HN, SQ], MMDT, kind="ExternalInput")
    v1 = nc.dram_tensor("v1", [pairs, SQ, HN + 1], MMDT, kind="ExternalInput")
    tri = nc.dram_tensor("tri", [128, 128], MMDT, kind="ExternalInput")
    out = nc.dram_tensor("ctxu", [pairs, HN + 1, SQ], F32, kind="ExternalOutput")

    with tile.TileContext(nc) as tc:
        with (
            tc.tile_pool(name="consts", bufs=1) as consts,
            tc.tile_pool(name="qk", bufs=2) as qkpool,
            tc.tile_pool(name="vp", bufs=2) as vpool,
            tc.tile_pool(name="exps", bufs=4) as epool,
            tc.tile_pool(name="outs", bufs=3) as opool,
            tc.tile_pool(name="spsum", bufs=2, space="PSUM") as spool,
            tc.tile_pool(name="cpsum", bufs=2, space="PSUM") as cpool,
        ):
            tri_t = consts.tile([128, 128], MMDT)
            nc.sync.dma_start(tri_t[:], tri[:])

            import contextlib

            loop_cm = (
                tc.For_i(0, loop_n, 1)
                if loop_n is not None
                else contextlib.nullcontext()
            )
            with loop_cm:
                _pair_body(
                    nc, pairs, repeat, nchunks, mask,
                    qt, kt, v1, out,
                    qkpool, vpool, epool, opool, spool, cpool, tri_t,
                )
    nc.finalize()
    return nc


def _pair_body(
    nc, pairs, repeat, nchunks, mask,
    qt, kt, v1, out,
    qkpool, vpool, epool, opool, spool, cpool, tri_t,
):
    MMDT = tri_t.dtype

    def emit_qk_group(s_ps, grp, j, qt_t, kt_t):
        # Scores for the blocks of one group, COMPACTED side by side:
        # slot for block i starts at the cumulative width so the whole
        # group is one contiguous span (one exp instruction, no garbage).
        placements = []
        c0 = 0
        for i in grp:
            off = max(0, 128 * i - CH * j)
            width = CH - off
            nc.tensor.matmul(
                s_ps[:, c0 : c0 + width],
                lhsT=kt_t[:, 128 * i : 128 * (i + 1)],
                rhs=qt_t[:, CH * j + off : CH * (j + 1)],
                start=True,
                stop=True,
            )
            placements.append((i, c0, off, width))
            c0 += width
        return placements, c0

    def plan_groups(j, nblocks):
        # Pack blocks into staging tiles of GROUP*CH elements.  A matmul
        # output may not cross a 512-elem PSUM bank boundary, so diagonal
        # blocks (widths 512/384/256/128) are ordered 512,384,128,256 --
        # with that order every span lands inside a bank.
        full = [i for i in range(nblocks) if 128 * i < CH * j]
        diag = [i for i in range(nblocks) if 128 * i >= CH * j]
        order = full + [diag[0], diag[1], diag[3], diag[2]]
        cap = GROUP * CH
        groups, cur, c0 = [], [], 0
        for i in order:
            off = max(0, 128 * i - CH * j)
            width = CH - off
            bank_rem = (-c0) % CH or CH
            if c0 + width > cap or (width > bank_rem):
                groups.append(cur)
                cur, c0 = [], 0
            cur.append(i)
            c0 += width
        if cur:
            groups.append(cur)
        return groups

    def load_pair(p, first):
        qt_t = qkpool.tile([HN, SQ], MMDT, tag="qt", name="qt_t")
        kt_t = qkpool.tile([HN, SQ], MMDT, tag="kt", name="kt_t")
        v1_t = vpool.tile([128, NBLK, HN + 1], MMDT, tag="v1", name="v1_t")
        if first:
            # split the very first loads so the first score group's
            # data lands early (cuts the pipeline-fill stall)
            kb = 512
            nc.sync.dma_start(qt_t[:, :CH], qt[p][:, :CH])
            nc.sync.dma_start(kt_t[:, :kb], kt[p][:, :kb])
            nc.sync.dma_start(qt_t[:, CH:], qt[p][:, CH:])
            nc.sync.dma_start(kt_t[:, kb:], kt[p][:, kb:])
        else:
            nc.sync.dma_start(qt_t[:], qt[p])
            nc.sync.dma_start(kt_t[:], kt[p])
        nc.sync.dma_start(v1_t[:], v1[p].rearrange("(i s) c -> s i c", s=128))
        return qt_t, kt_t, v1_t

    seq = [p for _ in range(repeat) for p in range(pairs)]

    # Build the flat list of group tasks.  Per-(pair,chunk) bookkeeping is
    # attached to the FIRST and LAST group of each chunk/pair so tile
    # allocation and copies/stores happen at the right flat positions.
    tasks = []
    for pi, p in enumerate(seq):
        for j in range(nchunks):
            nblocks = (j + 1) * (CH // 128)
            groups = plan_groups(j, nblocks)
            pv_seq = [i for grp in groups for i in grp]
            for gi, grp in enumerate(groups):
                tasks.append(
                    dict(
                        pi=pi, p=p, j=j, grp=grp,
                        first_of_chunk=(gi == 0),
                        last_of_chunk=(gi == len(groups) - 1),
                        first_of_pair=(gi == 0 and j == 0),
                        last_of_pair=(gi == len(groups) - 1 and j == nchunks - 1),
                        first_pv=pv_seq[0],
                        last_pv=pv_seq[-1],
                    )
                )

    # Software pipeline: PV of group g is emitted after QK of group g+2 so
    # the in-order PE never reaches a PV whose exp hasn't long finished;
    # each chunk's ctx leaves PSUM via DVE copy + its own store DMA.
    PV_DEPTH = 2
    state: dict = {}
    pend_pv: list = []

    def emit_pv(t):
        for i, c0, off, width in t["placements"]:
            nc.tensor.matmul(
                t["ctx_ps"][:, off:CH],
                lhsT=t["v1_t"][:, i, :],
                rhs=t["exps_t"][:, c0 : c0 + width],
                start=(i == t["first_pv"]),
                stop=(i == t["last_pv"]),
            )
        if t["last_of_chunk"]:
            j = t["j"]
            osb = opool.tile([HN + 1, CH], F32, tag="osb", name="osb")
            nc.vector.tensor_copy(osb[:], t["ctx_ps"][:])
            nc.sync.dma_start(out[t["p"]][:, CH * j : CH * (j + 1)], osb[:])

    for t in tasks:
        if t["first_of_pair"]:
            # tiles for this pair were prefetched one pair ago; issue the
            # NEXT pair's loads now so its QK never waits on DMA
            if t["pi"] == 0:
                state["tiles"] = load_pair(t["p"], True)
            else:
                state["tiles"] = state.pop("tiles_next")
            if t["pi"] + 1 < len(seq):
                state["tiles_next"] = load_pair(seq[t["pi"] + 1], False)
        qt_t, kt_t, v1_t = state["tiles"]
        if t["first_of_chunk"]:
            state["ctx_ps"] = cpool.tile([HN + 1, CH], F32, tag="ctx", name="ctx_ps")
        t["v1_t"], t["ctx_ps"] = v1_t, state["ctx_ps"]

        s_ps = spool.tile([128, GROUP * CH], F32, tag="s")
        t["placements"], total_w = emit_qk_group(s_ps, t["grp"], t["j"], qt_t, kt_t)
        t["exps_t"] = epool.tile([128, GROUP * CH], MMDT, tag="e", name="exps_t")
        nc.scalar.activation(
            t["exps_t"][:, :total_w],
            s_ps[:, :total_w],
            mybir.ActivationFunctionType.Exp,
            scale=0.125,
        )
        for i, c0, off, width in t["placements"]:
            if mask and 128 * i >= CH * t["j"]:
                # diagonal block: zero the upper triangle
                nc.vector.tensor_mul(
                    t["exps_t"][:, c0 : c0 + 128],
                    t["exps_t"][:, c0 : c0 + 128],
                    tri_t[:],
                )
        if len(pend_pv) >= PV_DEPTH:
            emit_pv(pend_pv.pop(0))
        pend_pv.append(t)

    while pend_pv:
        emit_pv(pend_pv.pop(0))


def prep_inputs(q: np.ndarray, k: np.ndarray, v: np.ndarray, mm_dtype=None):
    """Full [sq, b, np, hn] tensors -> per-pair device layouts."""
    npdt = mybir.dt.np(MM_DTYPE if mm_dtype is None else mm_dtype)
    q = np.asarray(q, dtype=np.float32)
    k = np.asarray(k, dtype=np.float32)
    v = np.asarray(v, dtype=np.float32)
    # [sq, b, np, hn] -> [b*np (pair), hn, sq]
    qt = np.ascontiguousarray(
        q.transpose(1, 2, 3, 0).reshape(PAIRS_TOTAL, HN, SQ).astype(npdt)
    )
    kt = np.ascontiguousarray(
        k.transpose(1, 2, 3, 0).reshape(PAIRS_TOTAL, HN, SQ).astype(npdt)
    )
    # [sq, b, np, hn] -> [pair, sq, hn] with ones column appended
    vr = v.transpose(1, 2, 0, 3).reshape(PAIRS_TOTAL, SQ, HN)
    v1 = np.concatenate(
        [vr, np.ones((PAIRS_TOTAL, SQ, 1), dtype=np.float32)], axis=2
    )
    v1 = np.ascontiguousarray(v1.astype(npdt))
    # exps is [sk (partition), q (free)]; keep iff q >= sk:
    # tri[s, c] = 1 where c >= s, which is exactly np.triu.
    tri = np.ascontiguousarray(np.triu(np.ones((128, 128), dtype=np.float32)).astype(npdt))
    return qt, kt, v1, tri


def postprocess(ctxu: np.ndarray) -> np.ndarray:
    """[pairs_total, 65, sq] unnormalized -> [sq, b, np*hn]."""
    ctx = ctxu[:, :HN, :] / ctxu[:, HN : HN + 1, :]
    # [pair, hn, sq] -> [sq, b, np, hn] -> [sq, b, np*hn]
    ctx = ctx.reshape(B, NP, HN, SQ).transpose(3, 0, 1, 2)
    return np.ascontiguousarray(ctx.reshape(SQ, B, NP * HN)).astype(np.float32)


_NC_CACHE: dict = {}


def kernel(query_layer, key_layer, value_layer, attention_mask=None, **_ignored):
    from concourse.bass_utils import run_bass_kernel_spmd

    qt, kt, v1, tri = prep_inputs(query_layer, key_layer, value_layer)

    if "nc" not in _NC_CACHE:
        _NC_CACHE["nc"] = build_attention_module(PAIRS)
    nc = _NC_CACHE["nc"]

    in_maps = []
    for c in range(N_CORES):
        sl = slice(c * PAIRS, (c + 1) * PAIRS)
        in_maps.append(
            {"qt": qt[sl], "kt": kt[sl], "v1": v1[sl], "tri": tri}
        )
    try:
        res = run_bass_kernel_spmd(nc, in_maps, core_ids=list(range(N_CORES)))
    except Exception:
        # rare transient device error: retry once
        res = run_bass_kernel_spmd(nc, in_maps, core_ids=list(range(N_CORES)))
    ctxu = np.concatenate([r["ctxu"] for r in res.results], axis=0)
    return postprocess(ctxu)


# revision 15
# speedup vs baseline: 1.0174x; 1.0174x over previous
"""Causal multi-head attention (CoreAttention) for Trainium2, 8 NeuronCores.

Strategy
--------
The problem is 64 independent (batch, head) attention instances of
[sq=2048, hn=64].  We shard them 8-per-core (tensor-parallel over heads x
data-parallel over batch) -- fully data parallel, no collectives.

Host-side (shard prep): Q and K are pre-transposed to [pair, hn, sq] and V
gets a ones-column appended ([pair, sq, 65]), all cast to fp16 (hw-measured
rel err 3.9e-4 on the max-err/scale metric), so that on-chip:

  S^T[sk_blk, q]   = matmul(lhsT=K^T[:, blk], rhs=Q^T[:, q_chunk])    (K=hn=64)
  E = exp(S^T / 8) via ScalarE straight out of PSUM, fp16 out
  causal triangle of diagonal blocks zeroed with one DVE multiply
  ctx^T[65, q]    += matmul(lhsT=[V|1][blk], rhs=E[blk])              (K=sk=128)

ctx^T row 64 is the softmax denominator; the final division and the
transpose back to [sq, b, np*hn] happen on the host.  Skipping the max
subtraction is safe: scores/8 ~ N(0,1), |s|<~7, exp is far from overflow
even in fp16, and softmax is shift invariant so the result matches.

Causality: sk blocks strictly above the diagonal are never computed;
diagonal-band matmuls restrict their q columns to the valid range, and
their score spans are COMPACTED side by side in the PSUM staging tile
(ordered so no matmul output crosses a 512-elem PSUM bank) so each group
of blocks needs exactly one ScalarE exp instruction.

Schedule: a flat software pipeline over all (pair, chunk, group) tasks.
PV of group g is emitted after QK of group g+2 so the in-order PE never
head-of-line blocks on an exp; pair inputs are DMA-prefetched one pair
ahead; each chunk's ctx leaves PSUM via DVE copy + its own store DMA.

Per core both engine floors bind at ~116 us (ACT: 8*17408 exp columns
@1.2GHz; PE: 2*8*17408 stream cycles @2.4GHz); measured 117.2 us/iter.
"""

import os
import sys

import numpy as np

if "/opt/trn_rl_repo" not in sys.path:
    sys.path.insert(0, "/opt/trn_rl_repo")

import concourse.bass as bass
import concourse.mybir as mybir
import concourse.tile as tile
from concourse import bacc

SQ, B, NP, HN = 2048, 4, 16, 64
N_CORES = 8
PAIRS_TOTAL = B * NP            # 64 (b, h) instances
PAIRS = PAIRS_TOTAL // N_CORES  # 8 per core
CH = 512                        # q chunk (one PSUM bank of fp32)
NBLK = SQ // 128                # 16 sk blocks
GROUP = 3                       # sk blocks per PSUM score-staging tile
F32 = mybir.dt.float32
MM_DTYPE = mybir.dt.float16     # matmul operand dtype (weights + streams)
EXP_BIAS = -8.317766            # -12*ln2: keeps fp16 exps/denominators in
                                # range; softmax shift-invariance cancels it


def build_attention_module(
    pairs: int = PAIRS,
    nchunks: int = SQ // CH,
    mask: bool = True,
    repeat: int = 1,
    mm_dtype=None,
    loop_n: int | None = None,
) -> bass.Bass:
    MMDT = MM_DTYPE if mm_dtype is None else mm_dtype
    nc = bacc.Bacc(trn_type="TRN2")
    qt = nc.dram_tensor("qt", [pairs, HN, SQ], MMDT, kind="ExternalInput")
    kt = nc.dram_tensor("kt", [pairs, HN, SQ], MMDT, kind="ExternalInput")
    v1 = nc.dram_tensor("v1", [pairs, SQ, HN + 1], MMDT, kind="ExternalInput")
    tri = nc.dram_tensor("tri", [128, 128], MMDT, kind="ExternalInput")
    ebias = nc.dram_tensor("ebias", [128, 1], F32, kind="ExternalInput")
    out = nc.dram_tensor("ctxu", [pairs, HN + 1, SQ], MMDT, kind="ExternalOutput")

    with tile.TileContext(nc) as tc:
        with (
            tc.tile_pool(name="consts", bufs=1) as consts,
            tc.tile_pool(name="qk", bufs=2) as qkpool,
            tc.tile_pool(name="vp", bufs=2) as vpool,
            tc.tile_pool(name="exps", bufs=4) as epool,
            tc.tile_pool(name="outs", bufs=3) as opool,
            tc.tile_pool(name="spsum", bufs=2, space="PSUM") as spool,
            tc.tile_pool(name="cpsum", bufs=2, space="PSUM") as cpool,
        ):
            tri_t = consts.tile([128, 128], MMDT)
            nc.sync.dma_start(tri_t[:], tri[:])
            ebias_t = consts.tile([128, 1], F32)
            nc.sync.dma_start(ebias_t[:], ebias[:])

            import contextlib

            loop_cm = (
                tc.For_i(0, loop_n, 1)
                if loop_n is not None
                else contextlib.nullcontext()
            )
            with loop_cm:
                _pair_body(
                    nc, pairs, repeat, nchunks, mask,
                    qt, kt, v1, out,
                    qkpool, vpool, epool, opool, spool, cpool, tri_t,
                    ebias_t,
                )
    nc.finalize()
    return nc


def _pair_body(
    nc, pairs, repeat, nchunks, mask,
    qt, kt, v1, out,
    qkpool, vpool, epool, opool, spool, cpool, tri_t,
    ebias_t,
):
    MMDT = tri_t.dtype

    def emit_qk_group(s_ps, grp, j, qt_t, kt_t):
        # Scores for the blocks of one group, COMPACTED side by side:
        # slot for block i starts at the cumulative width so the whole
        # group is one contiguous span (one exp instruction, no garbage).
        placements = []
        c0 = 0
        for i in grp:
            off = max(0, 128 * i - CH * j)
            width = CH - off
            nc.tensor.matmul(
                s_ps[:, c0 : c0 + width],
                lhsT=kt_t[:, 128 * i : 128 * (i + 1)],
                rhs=qt_t[:, CH * j + off : CH * (j + 1)],
                start=True,
                stop=True,
            )
            placements.append((i, c0, off, width))
            c0 += width
        return placements, c0

    def plan_groups(j, nblocks):
        # Pack blocks into staging tiles of GROUP*CH elements.  A matmul
        # output may not cross a 512-elem PSUM bank boundary, so diagonal
        # blocks (widths 512/384/256/128) are ordered 512,384,128,256 --
        # with that order every span lands inside a bank.
        full = [i for i in range(nblocks) if 128 * i < CH * j]
        diag = [i for i in range(nblocks) if 128 * i >= CH * j]
        order = full + [diag[0], diag[1], diag[3], diag[2]]
        cap = GROUP * CH
        groups, cur, c0 = [], [], 0
        for i in order:
            off = max(0, 128 * i - CH * j)
            width = CH - off
            bank_rem = (-c0) % CH or CH
            if c0 + width > cap or (width > bank_rem):
                groups.append(cur)
                cur, c0 = [], 0
            cur.append(i)
            c0 += width
        if cur:
            groups.append(cur)
        return groups

    def load_pair(p, first):
        qt_t = qkpool.tile([HN, SQ], MMDT, tag="qt", name="qt_t")
        kt_t = qkpool.tile([HN, SQ], MMDT, tag="kt", name="kt_t")
        v1_t = vpool.tile([128, NBLK, HN + 1], MMDT, tag="v1", name="v1_t")
        if first:
            # split the very first loads so the first score group's
            # data lands early (cuts the pipeline-fill stall)
            kb = 512
            nc.sync.dma_start(qt_t[:, :CH], qt[p][:, :CH])
            nc.sync.dma_start(kt_t[:, :kb], kt[p][:, :kb])
            nc.sync.dma_start(qt_t[:, CH:], qt[p][:, CH:])
            nc.sync.dma_start(kt_t[:, kb:], kt[p][:, kb:])
        else:
            nc.sync.dma_start(qt_t[:], qt[p])
            nc.sync.dma_start(kt_t[:], kt[p])
        nc.sync.dma_start(v1_t[:], v1[p].rearrange("(i s) c -> s i c", s=128))
        return qt_t, kt_t, v1_t

    seq = [p for _ in range(repeat) for p in range(pairs)]

    # Build the flat list of group tasks.  Per-(pair,chunk) bookkeeping is
    # attached to the FIRST and LAST group of each chunk/pair so tile
    # allocation and copies/stores happen at the right flat positions.
    tasks = []
    for pi, p in enumerate(seq):
        for j in range(nchunks):
            nblocks = (j + 1) * (CH // 128)
            groups = plan_groups(j, nblocks)
            pv_seq = [i for grp in groups for i in grp]
            for gi, grp in enumerate(groups):
                tasks.append(
                    dict(
                        pi=pi, p=p, j=j, grp=grp,
                        first_of_chunk=(gi == 0),
                        last_of_chunk=(gi == len(groups) - 1),
                        first_of_pair=(gi == 0 and j == 0),
                        last_of_pair=(gi == len(groups) - 1 and j == nchunks - 1),
                        first_pv=pv_seq[0],
                        last_pv=pv_seq[-1],
                    )
                )

    # Software pipeline: PV of group g is emitted after QK of group g+2 so
    # the in-order PE never reaches a PV whose exp hasn't long finished;
    # each chunk's ctx leaves PSUM via DVE copy + its own store DMA.
    PV_DEPTH = 2
    state: dict = {}
    pend_pv: list = []

    def emit_pv(t):
        for i, c0, off, width in t["placements"]:
            nc.tensor.matmul(
                t["ctx_ps"][:, off:CH],
                lhsT=t["v1_t"][:, i, :],
                rhs=t["exps_t"][:, c0 : c0 + width],
                start=(i == t["first_pv"]),
                stop=(i == t["last_pv"]),
            )
        if t["last_of_chunk"]:
            j = t["j"]
            osb = opool.tile([HN + 1, CH], MMDT, tag="osb", name="osb")
            nc.vector.tensor_copy(osb[:], t["ctx_ps"][:])
            nc.sync.dma_start(out[t["p"]][:, CH * j : CH * (j + 1)], osb[:])

    for t in tasks:
        if t["first_of_pair"]:
            # tiles for this pair were prefetched one pair ago; issue the
            # NEXT pair's loads now so its QK never waits on DMA
            if t["pi"] == 0:
                state["tiles"] = load_pair(t["p"], True)
            else:
                state["tiles"] = state.pop("tiles_next")
            if t["pi"] + 1 < len(seq):
                state["tiles_next"] = load_pair(seq[t["pi"] + 1], False)
        qt_t, kt_t, v1_t = state["tiles"]
        if t["first_of_chunk"]:
            state["ctx_ps"] = cpool.tile([HN + 1, CH], F32, tag="ctx", name="ctx_ps")
        t["v1_t"], t["ctx_ps"] = v1_t, state["ctx_ps"]

        s_ps = spool.tile([128, GROUP * CH], F32, tag="s")
        t["placements"], total_w = emit_qk_group(s_ps, t["grp"], t["j"], qt_t, kt_t)
        t["exps_t"] = epool.tile([128, GROUP * CH], MMDT, tag="e", name="exps_t")
        nc.scalar.activation(
            t["exps_t"][:, :total_w],
            s_ps[:, :total_w],
            mybir.ActivationFunctionType.Exp,
            scale=0.125,
            bias=ebias_t[:],
        )
        for i, c0, off, width in t["placements"]:
            if mask and 128 * i >= CH * t["j"]:
                # diagonal block: zero the upper triangle
                nc.vector.tensor_mul(
                    t["exps_t"][:, c0 : c0 + 128],
                    t["exps_t"][:, c0 : c0 + 128],
                    tri_t[:],
                )
        if len(pend_pv) >= PV_DEPTH:
            emit_pv(pend_pv.pop(0))
        pend_pv.append(t)

    while pend_pv:
        emit_pv(pend_pv.pop(0))


def prep_inputs(q: np.ndarray, k: np.ndarray, v: np.ndarray, mm_dtype=None):
    """Full [sq, b, np, hn] tensors -> per-pair device layouts."""
    npdt = mybir.dt.np(MM_DTYPE if mm_dtype is None else mm_dtype)
    q = np.asarray(q, dtype=np.float32)
    k = np.asarray(k, dtype=np.float32)
    v = np.asarray(v, dtype=np.float32)
    # [sq, b, np, hn] -> [b*np (pair), hn, sq]
    qt = np.ascontiguousarray(
        q.transpose(1, 2, 3, 0).reshape(PAIRS_TOTAL, HN, SQ).astype(npdt)
    )
    kt = np.ascontiguousarray(
        k.transpose(1, 2, 3, 0).reshape(PAIRS_TOTAL, HN, SQ).astype(npdt)
    )
    # [sq, b, np, hn] -> [pair, sq, hn] with ones column appended
    vr = v.transpose(1, 2, 0, 3).reshape(PAIRS_TOTAL, SQ, HN)
    v1 = np.concatenate(
        [vr, np.ones((PAIRS_TOTAL, SQ, 1), dtype=np.float32)], axis=2
    )
    v1 = np.ascontiguousarray(v1.astype(npdt))
    # exps is [sk (partition), q (free)]; keep iff q >= sk:
    # tri[s, c] = 1 where c >= s, which is exactly np.triu.
    tri = np.ascontiguousarray(np.triu(np.ones((128, 128), dtype=np.float32)).astype(npdt))
    ebias = np.full((128, 1), EXP_BIAS, dtype=np.float32)
    return qt, kt, v1, tri, ebias


def postprocess(ctxu: np.ndarray) -> np.ndarray:
    """[pairs_total, 65, sq] unnormalized -> [sq, b, np*hn]."""
    ctxu = np.asarray(ctxu, dtype=np.float32)
    ctx = ctxu[:, :HN, :] / ctxu[:, HN : HN + 1, :]
    # [pair, hn, sq] -> [sq, b, np, hn] -> [sq, b, np*hn]
    ctx = ctx.reshape(B, NP, HN, SQ).transpose(3, 0, 1, 2)
    return np.ascontiguousarray(ctx.reshape(SQ, B, NP * HN)).astype(np.float32)


_NC_CACHE: dict = {}


def kernel(query_layer, key_layer, value_layer, attention_mask=None, **_ignored):
    from concourse.bass_utils import run_bass_kernel_spmd

    qt, kt, v1, tri, ebias = prep_inputs(query_layer, key_layer, value_layer)

    if "nc" not in _NC_CACHE:
        _NC_CACHE["nc"] = build_attention_module(PAIRS)
    nc = _NC_CACHE["nc"]

    in_maps = []
    for c in range(N_CORES):
        sl = slice(c * PAIRS, (c + 1) * PAIRS)
        in_maps.append(
            {"qt": qt[sl], "kt": kt[sl], "v1": v1[sl], "tri": tri, "ebias": ebias}
        )
    try:
        res = run_bass_kernel_spmd(nc, in_maps, core_ids=list(range(N_CORES)))
    except Exception:
        # rare transient device error: retry once
        res = run_bass_kernel_spmd(nc, in_maps, core_ids=list(range(N_CORES)))
    ctxu = np.concatenate([r["ctxu"] for r in res.results], axis=0)
    return postprocess(ctxu)


# revision 17
# speedup vs baseline: 1.1385x; 1.1190x over previous
"""Causal multi-head attention (CoreAttention) for Trainium2, 8 NeuronCores.

Strategy
--------
The problem is 64 independent (batch, head) attention instances of
[sq=2048, hn=64].  We shard them 8-per-core (tensor-parallel over heads x
data-parallel over batch) -- fully data parallel, no collectives.

Host-side (shard prep): Q and K are pre-transposed to [pair, hn, sq] and V
gets a ones-column appended ([pair, sq, 65]), all cast to fp16 (hw-measured
rel err 3.9e-4 on the max-err/scale metric), so that on-chip:

  S^T[sk_blk, q]   = matmul(lhsT=K^T[:, blk], rhs=Q^T[:, q_chunk])    (K=hn=64)
  E = exp(S^T / 8) via ScalarE straight out of PSUM, fp16 out
  causal triangle of diagonal blocks zeroed with one DVE multiply
  ctx^T[65, q]    += matmul(lhsT=[V|1][blk], rhs=E[blk])              (K=sk=128)

ctx^T row 64 is the softmax denominator; the final division and the
transpose back to [sq, b, np*hn] happen on the host.  Skipping the max
subtraction is safe: scores/8 ~ N(0,1), |s|<~7, exp is far from overflow
even in fp16, and softmax is shift invariant so the result matches.

Causality: sk blocks strictly above the diagonal are never computed;
diagonal-band matmuls restrict their q columns to the valid range, and
their score spans are COMPACTED side by side in the PSUM staging tile
(ordered so no matmul output crosses a 512-elem PSUM bank) so each group
of blocks needs exactly one ScalarE exp instruction.

Schedule: a flat software pipeline over all (pair, chunk, group) tasks.
PV of group g is emitted after QK of group g+2 so the in-order PE never
head-of-line blocks on an exp; pair inputs are DMA-prefetched one pair
ahead; each chunk's ctx leaves PSUM via DVE copy + its own store DMA.

Per core both engine floors bind at ~116 us (ACT: 8*17408 exp columns
@1.2GHz; PE: 2*8*17408 stream cycles @2.4GHz); measured 117.2 us/iter on
a quiet device.  The shared device is bimodal (~1.55x slower when HBM is
contended), so exps carry a -12*ln2 bias (shift-invariant for softmax) to
keep the unnormalized ctx/denominator in fp16 range and halve the store
traffic; fp32r->fp16 everywhere cut input traffic 2x as well.
"""

import os
import sys

import numpy as np

if "/opt/trn_rl_repo" not in sys.path:
    sys.path.insert(0, "/opt/trn_rl_repo")

import concourse.bass as bass
import concourse.mybir as mybir
import concourse.tile as tile
from concourse import bacc

SQ, B, NP, HN = 2048, 4, 16, 64
N_CORES = 8
PAIRS_TOTAL = B * NP            # 64 (b, h) instances
PAIRS = PAIRS_TOTAL // N_CORES  # 8 per core
CH = 512                        # q chunk (one PSUM bank of fp32)
NBLK = SQ // 128                # 16 sk blocks
GROUP = 3                       # sk blocks per PSUM score-staging tile
F32 = mybir.dt.float32
MM_DTYPE = mybir.dt.float16     # matmul operand dtype (weights + streams)
EXP_BIAS = -8.317766            # -12*ln2: keeps fp16 exps/denominators in
                                # range; softmax shift-invariance cancels it


def build_attention_module(
    pairs: int = PAIRS,
    nchunks: int = SQ // CH,
    mask: bool = True,
    repeat: int = 1,
    mm_dtype=None,
    loop_n: int | None = None,
) -> bass.Bass:
    MMDT = MM_DTYPE if mm_dtype is None else mm_dtype
    nc = bacc.Bacc(trn_type="TRN2")
    qt = nc.dram_tensor("qt", [pairs, HN, SQ], MMDT, kind="ExternalInput")
    kt = nc.dram_tensor("kt", [pairs, HN, SQ], MMDT, kind="ExternalInput")
    v1 = nc.dram_tensor("v1", [pairs, 128, NBLK, HN + 1], MMDT, kind="ExternalInput")
    tri = nc.dram_tensor("tri", [128, 128], MMDT, kind="ExternalInput")
    ebias = nc.dram_tensor("ebias", [128, 1], F32, kind="ExternalInput")
    out = nc.dram_tensor("ctxu", [pairs, HN + 1, SQ], MMDT, kind="ExternalOutput")

    with tile.TileContext(nc) as tc:
        with (
            tc.tile_pool(name="consts", bufs=1) as consts,
            tc.tile_pool(name="qk", bufs=2) as qkpool,
            tc.tile_pool(name="vp", bufs=2) as vpool,
            tc.tile_pool(name="exps", bufs=4) as epool,
            tc.tile_pool(name="outs", bufs=3) as opool,
            tc.tile_pool(name="spsum", bufs=2, space="PSUM") as spool,
            tc.tile_pool(name="cpsum", bufs=2, space="PSUM") as cpool,
        ):
            tri_t = consts.tile([128, 128], MMDT)
            nc.sync.dma_start(tri_t[:], tri[:])
            ebias_t = consts.tile([128, 1], F32)
            nc.sync.dma_start(ebias_t[:], ebias[:])

            import contextlib

            loop_cm = (
                tc.For_i(0, loop_n, 1)
                if loop_n is not None
                else contextlib.nullcontext()
            )
            with loop_cm:
                _pair_body(
                    nc, pairs, repeat, nchunks, mask,
                    qt, kt, v1, out,
                    qkpool, vpool, epool, opool, spool, cpool, tri_t,
                    ebias_t,
                )
    nc.finalize()
    return nc


def _pair_body(
    nc, pairs, repeat, nchunks, mask,
    qt, kt, v1, out,
    qkpool, vpool, epool, opool, spool, cpool, tri_t,
    ebias_t,
):
    MMDT = tri_t.dtype

    def emit_qk_group(s_ps, grp, j, qt_t, kt_t):
        # Scores for the blocks of one group, COMPACTED side by side:
        # slot for block i starts at the cumulative width so the whole
        # group is one contiguous span (one exp instruction, no garbage).
        placements = []
        c0 = 0
        for i in grp:
            off = max(0, 128 * i - CH * j)
            width = CH - off
            nc.tensor.matmul(
                s_ps[:, c0 : c0 + width],
                lhsT=kt_t[:, 128 * i : 128 * (i + 1)],
                rhs=qt_t[:, CH * j + off : CH * (j + 1)],
                start=True,
                stop=True,
            )
            placements.append((i, c0, off, width))
            c0 += width
        return placements, c0

    def plan_groups(j, nblocks):
        # Pack blocks into staging tiles of GROUP*CH elements.  A matmul
        # output may not cross a 512-elem PSUM bank boundary, so diagonal
        # blocks (widths 512/384/256/128) are ordered 512,384,128,256 --
        # with that order every span lands inside a bank.
        full = [i for i in range(nblocks) if 128 * i < CH * j]
        diag = [i for i in range(nblocks) if 128 * i >= CH * j]
        order = full + [diag[0], diag[1], diag[3], diag[2]]
        cap = GROUP * CH
        groups, cur, c0 = [], [], 0
        for i in order:
            off = max(0, 128 * i - CH * j)
            width = CH - off
            bank_rem = (-c0) % CH or CH
            if c0 + width > cap or (width > bank_rem):
                groups.append(cur)
                cur, c0 = [], 0
            cur.append(i)
            c0 += width
        if cur:
            groups.append(cur)
        return groups

    def load_pair(p, first):
        qt_t = qkpool.tile([HN, SQ], MMDT, tag="qt", name="qt_t")
        kt_t = qkpool.tile([HN, SQ], MMDT, tag="kt", name="kt_t")
        v1_t = vpool.tile([128, NBLK, HN + 1], MMDT, tag="v1", name="v1_t")
        if first:
            # split the very first loads so the first score group's
            # data lands early (cuts the pipeline-fill stall)
            kb = 512
            nc.sync.dma_start(qt_t[:, :CH], qt[p][:, :CH])
            nc.sync.dma_start(kt_t[:, :kb], kt[p][:, :kb])
            nc.sync.dma_start(qt_t[:, CH:], qt[p][:, CH:])
            nc.sync.dma_start(kt_t[:, kb:], kt[p][:, kb:])
        else:
            nc.sync.dma_start(qt_t[:], qt[p])
            nc.sync.dma_start(kt_t[:], kt[p])
        # v1 is host-prearranged to [128, nblk, 65]: one contiguous 2080B
        # line per partition instead of a 2048-descriptor SWDGE scatter
        nc.sync.dma_start(v1_t[:], v1[p])
        return qt_t, kt_t, v1_t

    seq = [p for _ in range(repeat) for p in range(pairs)]

    # Build the flat list of group tasks.  Per-(pair,chunk) bookkeeping is
    # attached to the FIRST and LAST group of each chunk/pair so tile
    # allocation and copies/stores happen at the right flat positions.
    tasks = []
    for pi, p in enumerate(seq):
        for j in range(nchunks):
            nblocks = (j + 1) * (CH // 128)
            groups = plan_groups(j, nblocks)
            pv_seq = [i for grp in groups for i in grp]
            for gi, grp in enumerate(groups):
                tasks.append(
                    dict(
                        pi=pi, p=p, j=j, grp=grp,
                        first_of_chunk=(gi == 0),
                        last_of_chunk=(gi == len(groups) - 1),
                        first_of_pair=(gi == 0 and j == 0),
                        last_of_pair=(gi == len(groups) - 1 and j == nchunks - 1),
                        first_pv=pv_seq[0],
                        last_pv=pv_seq[-1],
                    )
                )

    # Software pipeline: PV of group g is emitted after QK of group g+2 so
    # the in-order PE never reaches a PV whose exp hasn't long finished;
    # each chunk's ctx leaves PSUM via DVE copy + its own store DMA.
    PV_DEPTH = 2
    state: dict = {}
    pend_pv: list = []

    def emit_pv(t):
        for i, c0, off, width in t["placements"]:
            nc.tensor.matmul(
                t["ctx_ps"][:, off:CH],
                lhsT=t["v1_t"][:, i, :],
                rhs=t["exps_t"][:, c0 : c0 + width],
                start=(i == t["first_pv"]),
                stop=(i == t["last_pv"]),
            )
        if t["last_of_chunk"]:
            j = t["j"]
            osb = opool.tile([HN + 1, CH], MMDT, tag="osb", name="osb")
            nc.vector.tensor_copy(osb[:], t["ctx_ps"][:])
            nc.sync.dma_start(out[t["p"]][:, CH * j : CH * (j + 1)], osb[:])

    for t in tasks:
        if t["first_of_pair"]:
            # tiles for this pair were prefetched one pair ago; issue the
            # NEXT pair's loads now so its QK never waits on DMA
            if t["pi"] == 0:
                state["tiles"] = load_pair(t["p"], True)
            else:
                state["tiles"] = state.pop("tiles_next")
            if t["pi"] + 1 < len(seq):
                state["tiles_next"] = load_pair(seq[t["pi"] + 1], False)
        qt_t, kt_t, v1_t = state["tiles"]
        if t["first_of_chunk"]:
            state["ctx_ps"] = cpool.tile([HN + 1, CH], F32, tag="ctx", name="ctx_ps")
        t["v1_t"], t["ctx_ps"] = v1_t, state["ctx_ps"]

        s_ps = spool.tile([128, GROUP * CH], F32, tag="s")
        t["placements"], total_w = emit_qk_group(s_ps, t["grp"], t["j"], qt_t, kt_t)
        t["exps_t"] = epool.tile([128, GROUP * CH], MMDT, tag="e", name="exps_t")
        nc.scalar.activation(
            t["exps_t"][:, :total_w],
            s_ps[:, :total_w],
            mybir.ActivationFunctionType.Exp,
            scale=0.125,
            bias=ebias_t[:],
        )
        for i, c0, off, width in t["placements"]:
            if mask and 128 * i >= CH * t["j"]:
                # diagonal block: zero the upper triangle
                nc.vector.tensor_mul(
                    t["exps_t"][:, c0 : c0 + 128],
                    t["exps_t"][:, c0 : c0 + 128],
                    tri_t[:],
                )
        if len(pend_pv) >= PV_DEPTH:
            emit_pv(pend_pv.pop(0))
        pend_pv.append(t)

    while pend_pv:
        emit_pv(pend_pv.pop(0))


def prep_inputs(q: np.ndarray, k: np.ndarray, v: np.ndarray, mm_dtype=None):
    """Full [sq, b, np, hn] tensors -> per-pair device layouts."""
    npdt = mybir.dt.np(MM_DTYPE if mm_dtype is None else mm_dtype)
    q = np.asarray(q, dtype=np.float32)
    k = np.asarray(k, dtype=np.float32)
    v = np.asarray(v, dtype=np.float32)
    # [sq, b, np, hn] -> [b*np (pair), hn, sq]
    qt = np.ascontiguousarray(
        q.transpose(1, 2, 3, 0).reshape(PAIRS_TOTAL, HN, SQ).astype(npdt)
    )
    kt = np.ascontiguousarray(
        k.transpose(1, 2, 3, 0).reshape(PAIRS_TOTAL, HN, SQ).astype(npdt)
    )
    # [sq, b, np, hn] -> [pair, sq, hn] with ones column appended
    vr = v.transpose(1, 2, 0, 3).reshape(PAIRS_TOTAL, SQ, HN)
    v1 = np.concatenate(
        [vr, np.ones((PAIRS_TOTAL, SQ, 1), dtype=np.float32)], axis=2
    )
    # [pair, sq, 65] -> [pair, 128 (s), nblk (i), 65]: device loads this as
    # one contiguous line per partition
    v1 = v1.reshape(PAIRS_TOTAL, NBLK, 128, HN + 1).transpose(0, 2, 1, 3)
    v1 = np.ascontiguousarray(v1.astype(npdt))
    # exps is [sk (partition), q (free)]; keep iff q >= sk:
    # tri[s, c] = 1 where c >= s, which is exactly np.triu.
    tri = np.ascontiguousarray(np.triu(np.ones((128, 128), dtype=np.float32)).astype(npdt))
    ebias = np.full((128, 1), EXP_BIAS, dtype=np.float32)
    return qt, kt, v1, tri, ebias


def postprocess(ctxu: np.ndarray) -> np.ndarray:
    """[pairs_total, 65, sq] unnormalized -> [sq, b, np*hn]."""
    ctxu = np.asarray(ctxu, dtype=np.float32)
    ctx = ctxu[:, :HN, :] / ctxu[:, HN : HN + 1, :]
    # [pair, hn, sq] -> [sq, b, np, hn] -> [sq, b, np*hn]
    ctx = ctx.reshape(B, NP, HN, SQ).transpose(3, 0, 1, 2)
    return np.ascontiguousarray(ctx.reshape(SQ, B, NP * HN)).astype(np.float32)


_NC_CACHE: dict = {}


def kernel(query_layer, key_layer, value_layer, attention_mask=None, **_ignored):
    from concourse.bass_utils import run_bass_kernel_spmd

    qt, kt, v1, tri, ebias = prep_inputs(query_layer, key_layer, value_layer)

    if "nc" not in _NC_CACHE:
        _NC_CACHE["nc"] = build_attention_module(PAIRS)
    nc = _NC_CACHE["nc"]

    in_maps = []
    for c in range(N_CORES):
        sl = slice(c * PAIRS, (c + 1) * PAIRS)
        in_maps.append(
            {"qt": qt[sl], "kt": kt[sl], "v1": v1[sl], "tri": tri, "ebias": ebias}
        )
    try:
        res = run_bass_kernel_spmd(nc, in_maps, core_ids=list(range(N_CORES)))
    except Exception:
        # rare transient device error: retry once
        res = run_bass_kernel_spmd(nc, in_maps, core_ids=list(range(N_CORES)))
    ctxu = np.concatenate([r["ctxu"] for r in res.results], axis=0)
    return postprocess(ctxu)
